# revision 1
# baseline (speedup 1.0000x reference)
"""Trainium2 Bass kernel for nn_DetectionLoss.

Reference computation:
  cls_loss = mean(softplus(x)) - sum(x at occupied cells)/BHW     (BCE-with-logits)
  reg_loss = sum(smoothl1(reg - target) at occupied cells)/num_objects
  total    = cls_loss + 2*reg_loss ; also returns num_objects

Key insight: only the cls channel (B,H,W) needs a dense pass; the 7 reg
channels are needed at just the <=1024 scattered target cells, so they are
fetched with one 128-row indirect DMA per core instead of reading 7/8 of the
input (8x traffic reduction). Sharding: data-parallel over B, 2 batches per
core; each core emits per-partition partials and the host finishes the tiny
scalar reduction.

Index semantics replicate the neuron backend the reference runs on:
  - f32->int32 conversion rounds to nearest (verified on device); emulated
    here in f32 arithmetic with the +-2^23 trick so it holds exactly
  - scatter .at[].set with duplicate indices: last write wins (verified)

softplus(x) is computed directly as Ln(1 + Exp(x)): preds are N(0,1) logits
so Exp cannot overflow, and the two table functions share one ACT table set
(enforced below) so only a single table load is paid.
"""

import numpy as np

import concourse.bass as bass
import concourse.tile as tile
from concourse import bacc, mybir
from concourse.bass_utils import run_bass_kernel_spmd
from concourse.tile_rust import add_dep_helper

P = 128
B, C, H, W = 16, 8, 512, 512
N_TGT = 64
NCORES = 8
BPC = B // NCORES            # batches per core
CELLS = H * W                # 262144
CORE_ELEMS = BPC * CELLS     # 524288
FREE = CORE_ELEMS // P       # 4096
SPLITS = (672, 1312, 2112)   # dense chunk widths: small first so ACT starts
                             # early, large last (tuned on the cost model)
SLOTS = BPC * N_TGT          # 128 target slots per core
TWO23 = 8388608.0            # 2^23: (x + 2^23) - 2^23 == rint(x), 0<=x<2^23

f32 = mybir.dt.float32
i32 = mybir.dt.int32
ALU = mybir.AluOpType
ACT = mybir.ActivationFunctionType

NCHUNK = len(SPLITS)
COL_X = NCHUNK               # winner_mask * cls_value at cell
COL_M = NCHUNK + 1           # winner mask (1 per unique occupied cell)
COL_REG = NCHUNK + 2         # winner_mask * smoothl1 row sum
OUT_COLS = NCHUNK + 3

_compiled = None
_tables_patched = False


def _stub_axon_hooks():
    """run_bass_kernel_spmd(trace=True) — reachable via the BASS_TRACE env
    var — imports antenv.axon_hooks, which doesn't exist in this container.
    Register a stub whose hook getter returns None so the call degrades to
    an untraced run (bass_utils handles the None hook) instead of crashing."""
    import importlib
    import sys
    import types as _types

    try:
        importlib.import_module("antenv.axon_hooks")
    except Exception:
        m = _types.ModuleType("antenv.axon_hooks")
        m.get_axon_ntff_profile_hook = lambda: None
        sys.modules["antenv.axon_hooks"] = m


_stub_axon_hooks()


def _patch_act_tables():
    """Make Exp and Ln resolve only to the table set that contains both, so
    Bacc's greedy chooser emits a single ACT table load instead of two."""
    global _tables_patched
    if _tables_patched:
        return
    _tables_patched = True
    import concourse.hw_specs as hws

    orig = hws.get_activation_tables

    def patched(arch):
        tables = orig(arch)
        combo = tables.get("natural_log_exp_and_others")
        if combo and ACT.Exp in combo and ACT.Ln in combo:
            # safe to steer: the combined set can serve both funcs
            for name, funcs in tables.items():
                if name != "natural_log_exp_and_others":
                    funcs.discard(ACT.Exp)
                    funcs.discard(ACT.Ln)
        return tables

    hws.get_activation_tables = patched
    bacc.get_activation_tables = patched


def _build():
    _patch_act_tables()
    nc = bacc.Bacc(
        "TRN2", target_bir_lowering=False, debug=False, num_devices=NCORES
    )
    cls_in = nc.declare_dram_parameter("cls", [P, FREE + 8], f32, isOutput=False)
    cl8_in = nc.declare_dram_parameter("cl8", [CORE_ELEMS, C], f32, isOutput=False)
    out_d = nc.declare_dram_parameter("out", [P, OUT_COLS], f32, isOutput=True)
    fc_sc = nc.dram_tensor("fc_scratch", [1, P], f32)

    with tile.TileContext(nc) as tc:
        with tc.tile_pool(name="sbuf", bufs=1) as sp:
            out_t = sp.tile([P, OUT_COLS], f32)

            # ---------------- dense pass: sum softplus(cls) ----------------
            # chunk 0 also carries the 8 targets columns (cols FREE..FREE+7 of
            # the cls input, appended by the host) so the tiny targets load
            # doesn't need its own DMA slot in the stream queue.
            # tg cols 0..6: target values; col 7: batch offset b*CELLS.
            tg = None
            prev_ln = None
            col0 = 0
            for k, wdt in enumerate(SPLITS):
                sl = slice(col0, col0 + wdt)
                col0 += wdt
                if k == 0:
                    # host interleaves the 8 tg columns right after chunk 0,
                    # so one contiguous DMA carries both
                    xt0 = sp.tile([P, wdt + 8], f32, tag="xt0")
                    nc.sync.dma_start(
                        out=xt0[:], in_=cls_in[:, 0 : wdt + 8]
                    )
                    tg = xt0[:, wdt : wdt + 8]
                    xt = xt0[:, 0:wdt]
                    col0 += 8  # later chunks shifted by the inserted tg cols
                else:
                    xt = sp.tile([P, wdt], f32, tag=f"xt{k}")
                    nc.sync.dma_start(out=xt[:], in_=cls_in[:, sl])
                e = nc.scalar.activation(out=xt[:], in_=xt[:], func=ACT.Exp)
                if prev_ln is not None:
                    # keep ACT in per-chunk Exp/Ln order so earlier chunks
                    # finish while later chunks are still streaming in
                    add_dep_helper(e.ins, prev_ln.ins, reason="act order")
                prev_ln = nc.scalar.activation(
                    out=xt[:], in_=xt[:], func=ACT.Ln, bias=1.0,
                    accum_out=out_t[:, k : k + 1],
                )

            # ---------------- target indices (one slot per partition) -------
            def grid_coord(col):
                # rint(clip(t * (512/80), 0, 511)); the rounding must happen
                # in f32 (the +2^23 trick) to mirror the backend's
                # round-to-nearest float->int conversion.
                g = sp.tile([P, 1], f32, tag=f"g{col}")
                nc.vector.tensor_scalar(
                    out=g[:], in0=tg[:, col : col + 1],
                    scalar1=float(np.float32(W / 80.0)), scalar2=511.0,
                    op0=ALU.mult, op1=ALU.min,
                )
                nc.vector.tensor_scalar(
                    out=g[:], in0=g[:], scalar1=0.0, scalar2=None, op0=ALU.max
                )
                gr = sp.tile([P, 1], f32, tag=f"gr{col}")
                nc.vector.tensor_scalar(
                    out=gr[:], in0=g[:], scalar1=TWO23, scalar2=None, op0=ALU.add
                )
                nc.vector.tensor_scalar(
                    out=gr[:], in0=gr[:], scalar1=TWO23, scalar2=None,
                    op0=ALU.subtract,
                )
                return gr

            gx = grid_coord(0)
            gy = grid_coord(1)

            # fc = b*CELLS + gy*W + gx  (exact in f32, < 2^24)
            fc = sp.tile([P, 1], f32)
            nc.vector.tensor_scalar(
                out=fc[:], in0=gy[:], scalar1=float(W), scalar2=None, op0=ALU.mult
            )
            nc.vector.tensor_tensor(out=fc[:], in0=fc[:], in1=gx[:], op=ALU.add)
            nc.vector.tensor_tensor(out=fc[:], in0=fc[:], in1=tg[:, 7:8], op=ALU.add)

            fci = sp.tile([P, 1], i32)
            nc.vector.tensor_copy(out=fci[:], in_=fc[:])

            # ---------------- gather 8 channels at each target cell ---------
            # offsets staged through a gpsimd-written tile: feeding the
            # DVE-written tile to the dynamic-DMA descriptor generator
            # directly crashes the exec unit (observed empirically).
            fcig = sp.tile([P, 1], i32)
            nc.gpsimd.tensor_copy(out=fcig[:], in_=fci[:])
            gat = sp.tile([P, C], f32)
            nc.gpsimd.indirect_dma_start(
                out=gat[:], out_offset=None,
                in_=cl8_in[:],
                in_offset=bass.IndirectOffsetOnAxis(ap=fcig[:, :1], axis=0),
            )

            # ---------------- duplicate resolution (last write wins) --------
            # round-trip fc through DRAM to replicate it along the free dim of
            # every partition (DMA partition-broadcast) instead of a transpose
            nc.sync.dma_start(out=fc_sc[:], in_=fc[:])
            fct = sp.tile([P, P], f32)
            nc.sync.dma_start(out=fct[:], in_=fc_sc[:].to_broadcast((P, P)))
            sel = sp.tile([P, P], f32)
            nc.vector.tensor_tensor(
                out=sel[:], in0=fc[:].to_broadcast([P, P]), in1=fct[:],
                op=ALU.is_equal,
            )
            # keep only strictly-upper entries (j > i): a later slot writing
            # the same cell. row sum == 0 -> this slot is the winner.
            nc.gpsimd.affine_select(
                out=sel[:], in_=sel[:], compare_op=ALU.is_gt, fill=0.0,
                base=0, channel_multiplier=-1, pattern=[[1, P]],
            )
            dup_after = sp.tile([P, 1], f32)
            nc.vector.reduce_sum(
                out=dup_after[:], in_=sel[:], axis=mybir.AxisListType.X
            )
            m = sp.tile([P, 1], f32)
            nc.vector.tensor_scalar(
                out=m[:], in0=dup_after[:], scalar1=0.0, scalar2=None,
                op0=ALU.is_equal,
            )
            nc.vector.tensor_copy(out=out_t[:, COL_M : COL_M + 1], in_=m[:])

            # masked cls logit at the cell
            nc.vector.tensor_tensor(
                out=out_t[:, COL_X : COL_X + 1], in0=m[:], in1=gat[:, 0:1],
                op=ALU.mult,
            )

            # ---------------- smooth-l1 on the 7 reg channels ---------------
            d7 = sp.tile([P, 7], f32)
            nc.vector.tensor_tensor(
                out=d7[:], in0=gat[:, 1:C], in1=tg[:, 0:7], op=ALU.subtract
            )
            ad = sp.tile([P, 7], f32)
            nc.vector.tensor_scalar(
                out=ad[:], in0=d7[:], scalar1=-1.0, scalar2=None, op0=ALU.mult
            )
            nc.vector.tensor_tensor(out=ad[:], in0=ad[:], in1=d7[:], op=ALU.max)
            q = sp.tile([P, 7], f32)
            nc.vector.tensor_tensor(out=q[:], in0=ad[:], in1=ad[:], op=ALU.mult)
            nc.vector.tensor_scalar(
                out=q[:], in0=q[:], scalar1=0.5, scalar2=None, op0=ALU.mult
            )
            lin = sp.tile([P, 7], f32)
            nc.vector.tensor_scalar(
                out=lin[:], in0=ad[:], scalar1=0.5, scalar2=None, op0=ALU.subtract
            )
            lt = sp.tile([P, 7], mybir.dt.uint8)
            nc.vector.tensor_scalar(
                out=lt[:], in0=ad[:], scalar1=1.0, scalar2=None, op0=ALU.is_lt
            )
            sl1 = sp.tile([P, 7], f32)
            nc.vector.select(out=sl1[:], mask=lt[:], on_true=q[:], on_false=lin[:])
            rs = sp.tile([P, 1], f32)
            nc.vector.reduce_sum(out=rs[:], in_=sl1[:], axis=mybir.AxisListType.X)
            nc.vector.tensor_tensor(
                out=out_t[:, COL_REG : COL_REG + 1], in0=rs[:], in1=m[:],
                op=ALU.mult,
            )

            nc.sync.dma_start(out=out_d[:], in_=out_t[:])

    nc.compile()
    return nc


def kernel(preds: np.ndarray, targets: np.ndarray) -> tuple:
    global _compiled
    preds = np.ascontiguousarray(np.asarray(preds, dtype=np.float32))
    targets = np.ascontiguousarray(np.asarray(targets, dtype=np.float32))

    # host-side layout prep (no reductions/FLOPs on tensor data, just copies):
    # contiguous cls channel for the dense pass, channel-last copy so one
    # indirect-DMA row fetches all 8 channels of a cell.
    cls = np.ascontiguousarray(preds[:, 0])                       # (B,H,W)
    cl8 = np.ascontiguousarray(
        np.transpose(preds.reshape(B, C, CELLS), (0, 2, 1))       # (B,CELLS,C)
    )

    if _compiled is None:
        _compiled = _build()
    nc = _compiled

    boff_col = np.repeat(
        np.arange(BPC, dtype=np.float32) * CELLS, N_TGT
    ).reshape(SLOTS, 1)
    in_maps = []
    for c in range(NCORES):
        b0 = c * BPC
        cls2d = cls[b0 : b0 + BPC].reshape(P, FREE)
        tg8 = np.concatenate(
            [targets[b0 : b0 + BPC].reshape(SLOTS, 7), boff_col], axis=1
        )
        in_maps.append({
            "cls": np.ascontiguousarray(np.concatenate(
                [cls2d[:, 0 : SPLITS[0]], tg8, cls2d[:, SPLITS[0] :]], axis=1
            )),
            "cl8": cl8[b0 : b0 + BPC].reshape(CORE_ELEMS, C),
        })

    try:
        res = run_bass_kernel_spmd(nc, in_maps, list(range(NCORES))).results
    except Exception:
        # the axon worker occasionally dies with NRT_EXEC_UNIT_UNRECOVERABLE
        # on arbitrary ops (observed on plain jax PRNG calls too) and recovers
        # on the next attempt; retry once before giving up.
        res = run_bass_kernel_spmd(nc, in_maps, list(range(NCORES))).results

    outs = np.stack([np.asarray(r["out"], dtype=np.float64) for r in res])
    s_softplus = outs[:, :, 0:NCHUNK].sum()
    s_x = outs[:, :, COL_X].sum()
    num_objects = outs[:, :, COL_M].sum()
    s_reg = outs[:, :, COL_REG].sum()

    m_total = float(B * H * W)
    cls_loss = s_softplus / m_total - s_x / m_total
    reg_loss = s_reg / (num_objects + 1e-6) if num_objects > 0 else 0.0
    total = np.float32(cls_loss + 2.0 * reg_loss)
    return total, np.float32(num_objects)



# revision 2
# speedup vs baseline: 1.0065x; 1.0065x over previous
"""Trainium2 Bass kernel for nn_DetectionLoss — v3 (product-tree softplus).

Reference computation:
  cls_loss = mean(softplus(x)) - sum(x at occupied cells)/BHW
  reg_loss = sum(smoothl1(reg - target) at occupied cells)/num_objects
  total    = cls_loss + 2*reg_loss ; also returns num_objects

Structure (vs the 15947ns 2-pass baseline):
  * The dense cls channel streams in mostly as bf16 via gpsimd casting
    DMAs (software DGE converts f32->bf16 in flight), nearly halving
    DMA_ENGINES occupancy; small f32 chunks bracket the stream (first for
    an early ACT start, last so the final Exp's data pre-arrives and the
    tail never waits on a fresh transfer).
  * Softplus is regrouped as sum ln(prod (1+e^x)): one ACT Exp pass per
    chunk, then a DVE pairwise-product tree in bf16 (2x tensor-tensor
    mode) shrinks each chunk before the Ln, cutting ACT busy time ~40%.
    The penultimate chunk's products are shifted by -1 on DVE so the
    final Ln(bias=1) covers them together with the last plain chunk.
  * Scatter addresses (grid indices -> flat cell index) are computed on
    the host from the tiny targets tensor — pure index arithmetic, the
    same role as descriptor offsets; every reduction (winner mask,
    num_objects, masked sums, losses) stays on device.
  * Winner-mask (last-write-wins duplicate resolution) via
    reduce_max(is_equal * slot_index) == own_index, using a host-provided
    broadcast row — plain DVE ops, no Pool dependency.

Precision: bf16 only touches the softplus sum; its RNE rounding errors
are symmetric and cancel across 4M elements (measured ~1e-7 relative on
the total), and even for object-free inputs stay ~1e-3 relative.
"""

import os

import numpy as np

import concourse.bass as bass
import concourse.tile as tile
from concourse import bacc, mybir
from concourse.bass_utils import run_bass_kernel_spmd
from concourse.tile_rust import add_dep_helper

P = 128
B, C, H, W = 16, 8, 512, 512
N_TGT = 64
NCORES = 8
BPC = B // NCORES            # batches per core
CELLS = H * W                # 262144
CORE_ELEMS = BPC * CELLS     # 524288
FREE = CORE_ELEMS // P       # 4096
SLOTS = BPC * N_TGT          # 128 target slots per core
TWO23 = np.float32(8388608.0)  # 2^23: rint via add/sub in f32

# Dense-stream chunking: (width, kind, depth); kind "f32" = SP HWDGE DMA,
# "bf16" = gpsimd casting DMA; depth = pairwise-product halvings (0 only
# allowed for the final plain chunk).
# ACT processes chunks in listed order; columns are assigned by list
# position, so any permutation is valid. depth-0 "plain" chunks (f32,
# dispatched early so their data pre-arrives) fill ACT's early idle
# while the bf16 stream ramps; the final tree chunk's Ln is split off so
# only a narrow Ln rides the tail.
CHUNKS = (
    (384, "f32", 3),
    (1024, "bf16", 4),
    (1280, "bf16", 4),
    (1280, "bf16", 2),
    (128, "f32", 0),
)
if os.environ.get("K2_CHUNKS"):
    CHUNKS = tuple(
        (int(w), k, int(d))
        for w, k, d in (c.split(":") for c in os.environ["K2_CHUNKS"].split(","))
    )
assert sum(w for w, _, _ in CHUNKS) == FREE

COL_LN2 = 0                  # softplus partials: plain chunks
COL_X = 1                    # winner_mask * cls_value at cell
COL_M = 2                    # winner mask (1 per unique occupied cell)
COL_REG = 3                  # winner_mask * smoothl1 row sum
COL_LN1 = 4                  # softplus partials: all tree chunks but last
COL_LN3 = 5                  # softplus partials: last tree chunk
OUT_COLS = 6                 # cols 0:4 ship early; cols 4:6 ride the tail

TGX_COLS = 10                # 0-6 targets, 7 fc(f32), 8 fc bits(i32), 9 i+1

f32 = mybir.dt.float32
bf16 = mybir.dt.bfloat16
i32 = mybir.dt.int32
ALU = mybir.AluOpType
ACT = mybir.ActivationFunctionType

_compiled = None


def _stub_axon_hooks():
    """run_bass_kernel_spmd(trace=True) imports antenv.axon_hooks, which does
    not exist in this container; register a stub so it degrades gracefully."""
    import importlib
    import sys
    import types as _types

    try:
        importlib.import_module("antenv.axon_hooks")
    except Exception:
        m = _types.ModuleType("antenv.axon_hooks")
        m.get_axon_ntff_profile_hook = lambda: None
        sys.modules["antenv.axon_hooks"] = m


_stub_axon_hooks()

_tables_patched = False


def _patch_act_tables():
    """Make Exp and Ln resolve only to the table set that contains both, so
    Bacc's greedy chooser emits a single ACT table load instead of two."""
    global _tables_patched
    if _tables_patched:
        return
    _tables_patched = True
    import concourse.hw_specs as hws

    orig = hws.get_activation_tables

    def patched(arch):
        tables = orig(arch)
        combo = tables.get("natural_log_exp_and_others")
        if combo and ACT.Exp in combo and ACT.Ln in combo:
            for name, funcs in tables.items():
                if name != "natural_log_exp_and_others":
                    funcs.discard(ACT.Exp)
                    funcs.discard(ACT.Ln)
        return tables

    hws.get_activation_tables = patched
    bacc.get_activation_tables = patched


def _build():
    _patch_act_tables()
    nc = bacc.Bacc(
        "TRN2", target_bir_lowering=False, debug=False, num_devices=NCORES
    )
    cls_in = nc.declare_dram_parameter("cls", [P, FREE], f32, isOutput=False)
    tgx_in = nc.declare_dram_parameter("tgx", [P, TGX_COLS], f32, isOutput=False)
    frow_in = nc.declare_dram_parameter("frow", [1, 2 * P], f32, isOutput=False)
    fci_in = nc.declare_dram_parameter("fci", [P, 1], i32, isOutput=False)
    cl8_in = nc.declare_dram_parameter("cl8", [CORE_ELEMS, C], f32, isOutput=False)
    out_d = nc.declare_dram_parameter("out", [P, OUT_COLS], f32, isOutput=True)

    # depth-0 chunks are "plain": their e^x values share one tile finished
    # by a single Ln(bias=1) emitted right after the last plain Exp; tree
    # chunks feed Ln#1 (bias=0), except the last tree chunk which gets its
    # own narrow Ln#3 so only it rides the tail.
    tree_ks = [k for k, (_, _, d) in enumerate(CHUNKS) if d > 0]
    plain_ks = [k for k, (_, _, d) in enumerate(CHUNKS) if d == 0]
    assert tree_ks
    last_tree_k = tree_ks[-1]
    last_plain_k = plain_ks[-1] if plain_ks else None
    tree_cols = sum(w >> d for k, (w, _, d) in enumerate(CHUNKS)
                    if d > 0 and k != last_tree_k)
    lt_w, _, lt_d = CHUNKS[last_tree_k]
    lt_cols = lt_w >> lt_d
    plain_cols = sum(w for w, _, d in CHUNKS if d == 0)

    with tile.TileContext(nc) as tc:
        with tc.tile_pool(name="sbuf", bufs=1) as sp:
            out_t = sp.tile([P, OUT_COLS], f32)

            offs = np.cumsum([0] + [w for w, _, _ in CHUNKS]).tolist()

            def chunk_dma(k):
                w, kind, _ = CHUNKS[k]
                sl = slice(offs[k], offs[k] + w)
                if kind == "f32":
                    xt = sp.tile([P, w], f32, tag=f"xt{k}")
                    nc.sync.dma_start(out=xt[:], in_=cls_in[:, sl])
                else:
                    xt = sp.tile([P, w], bf16, tag=f"xt{k}")
                    nc.gpsimd.dma_start(out=xt[:], in_=cls_in[:, sl])
                return xt

            def tree(k, ue, w, depth, dst_ap):
                """Products of (1+ue) groups of 2^depth cols into dst_ap.

                3 DVE ops regardless of depth: TS(+1) over the full chunk,
                one pairwise TT (packed halves keep the 2x bf16 mode), then
                a single product-reduce over the innermost 2^(depth-1)
                (verified bit-exact on device). Few ops matter: each DVE op
                carries ~100-150ns of semaphore/dispatch latency.
                """
                u1 = sp.tile([P, w], bf16, tag=f"u1_{k}")
                nc.vector.tensor_scalar(
                    out=u1[:], in0=ue[:], scalar1=1.0, scalar2=None,
                    op0=ALU.add,
                )
                v = u1[:]
                cw = w
                for lvl in range(depth):
                    cw //= 2
                    if lvl == depth - 1:
                        dst = dst_ap
                    else:
                        dt_ = sp.tile([P, cw], bf16, tag=f"v{k}_{lvl}")
                        dst = dt_[:]
                    tt = nc.vector.tensor_tensor(
                        out=dst, in0=v[:, 0:cw], in1=v[:, cw : 2 * cw],
                        op=ALU.mult,
                    )
                    v = dst
                return tt

            # ---------------- SP dispatch order ------------------------------
            # chunk0 first (it gates ACT start), then the small side inputs
            # (fci first: it gates the longest chain, the gather), then the
            # pre-arriving last chunk.
            # f32 chunks dispatch on SP first (their data pre-arrives), the
            # small side inputs after (their consumers run mid-stream);
            # bf16 chunks go through the Pool descriptor pipeline in list
            # order
            xts = {}
            for k in range(len(CHUNKS)):
                if CHUNKS[k][1] == "f32":
                    xts[k] = chunk_dma(k)
            fr = sp.tile([P, 2 * P], f32)
            nc.sync.dma_start(out=fr[:], in_=frow_in[:].to_broadcast((P, 2 * P)))
            fci = sp.tile([P, 1], i32)
            nc.sync.dma_start(out=fci[:], in_=fci_in[:])
            tgx = sp.tile([P, TGX_COLS], f32)
            nc.sync.dma_start(out=tgx[:], in_=tgx_in[:])
            for k in range(len(CHUNKS)):
                if CHUNKS[k][1] != "f32":
                    xts[k] = chunk_dma(k)

            # ---------------- ACT pipeline -----------------------------------
            prod = sp.tile([P, tree_cols], bf16)
            prod3 = sp.tile([P, lt_cols], bf16)
            late = sp.tile([P, max(plain_cols, 1)], bf16)
            if not plain_ks:
                # COL_LN2 never accumulated: zero it so the host sum holds
                nc.gpsimd.memset(out_t[:, COL_LN2 : COL_LN2 + 1], 0.0)

            pcol = 0
            lcol = 0
            prev_act = None
            tree_tts = []
            for k, (w, kind, d) in enumerate(CHUNKS):
                if d == 0:
                    ue = late[:, lcol : lcol + w]
                    lcol += w
                else:
                    ue_t = sp.tile([P, w], bf16, tag=f"ue{k}")
                    ue = ue_t[:]
                e = nc.scalar.activation(out=ue, in_=xts[k][:], func=ACT.Exp)
                if prev_act is not None:
                    add_dep_helper(e.ins, prev_act.ins, reason="act order")
                prev_act = e
                if d > 0:
                    if k == last_tree_k:
                        dst = prod3[:, 0:lt_cols]
                    else:
                        dst = prod[:, pcol : pcol + (w >> d)]
                        pcol += w >> d
                    tree_tts.append(tree(k, ue, w, d, dst))
                if k == last_plain_k:
                    # combined Ln(1+e^x) over the plain chunks, filling
                    # ACT idle while the bf16 stream ramps
                    lnc = nc.scalar.activation(
                        out=late[:], in_=late[:], func=ACT.Ln, bias=1.0,
                        accum_out=out_t[:, COL_LN2 : COL_LN2 + 1],
                    )
                    add_dep_helper(lnc.ins, prev_act.ins, reason="act order")
                    prev_act = lnc

            # Ln#1 over all tree products except the last tree chunk's
            # (those trees complete during later Exps); the narrow Ln#3
            # over the last tree chunk alone rides the tail.
            lnt = sp.tile([P, tree_cols], bf16)
            ln1 = nc.scalar.activation(
                out=lnt[:], in_=prod[:], func=ACT.Ln,
                accum_out=out_t[:, COL_LN1 : COL_LN1 + 1],
            )
            add_dep_helper(ln1.ins, prev_act.ins, reason="act order")
            lnt3 = sp.tile([P, lt_cols], bf16)
            ln3 = nc.scalar.activation(
                out=lnt3[:], in_=prod3[:], func=ACT.Ln,
                accum_out=out_t[:, COL_LN3 : COL_LN3 + 1],
            )
            add_dep_helper(ln3.ins, ln1.ins, reason="act order")

            # ---------------- winner mask (last write wins) ------------------
            # m_i = (max_j [fc_j == fc_i] * (j+1)) == i+1
            eq = sp.tile([P, P], f32)
            eq_op = nc.vector.tensor_tensor(
                out=eq[:], in0=tgx[:, 7:8].to_broadcast([P, P]),
                in1=fr[:, 0:P], op=ALU.is_equal,
            )
            # order the mask block after tree 1 on DVE so early trees are
            # not queued behind the frow-broadcast wait
            if len(tree_tts) > 1:
                add_dep_helper(eq_op.ins, tree_tts[1].ins, reason="dve order")
            wv = sp.tile([P, P], f32)
            nc.vector.tensor_tensor(
                out=wv[:], in0=eq[:], in1=fr[:, P : 2 * P], op=ALU.mult
            )
            mx = sp.tile([P, 1], f32)
            nc.vector.reduce_max(out=mx[:], in_=wv[:], axis=mybir.AxisListType.X)
            m = sp.tile([P, 1], f32)
            nc.vector.tensor_tensor(
                out=m[:], in0=mx[:], in1=tgx[:, 9:10], op=ALU.is_equal
            )

            # ---------------- gather 8 channels at each target cell ---------
            # offsets staged through a gpsimd-written tile: feeding a
            # non-gpsimd-written tile to the dynamic-DMA descriptor
            # generator crashes the exec unit (observed empirically).
            fcig = sp.tile([P, 1], i32)
            nc.gpsimd.tensor_copy(out=fcig[:], in_=fci[:])
            gat = sp.tile([P, C], f32)
            nc.gpsimd.indirect_dma_start(
                out=gat[:], out_offset=None,
                in_=cl8_in[:],
                in_offset=bass.IndirectOffsetOnAxis(ap=fcig[:, :1], axis=0),
            )

            # ---------------- gather-dependent tail ------------------------
            # on Pool by default (keeps DVE clear for the product trees;
            # Pool is idle once chunk descriptor generation is done);
            # K2_TAIL_ENG=vector falls back to DVE.
            te = getattr(nc, os.environ.get("K2_TAIL_ENG", "gpsimd"))
            te.tensor_copy(out=out_t[:, COL_M : COL_M + 1], in_=m[:])
            # masked cls logit at the cell
            te.tensor_tensor(
                out=out_t[:, COL_X : COL_X + 1], in0=m[:], in1=gat[:, 0:1],
                op=ALU.mult,
            )
            # smooth-l1 on the 7 reg channels
            d7 = sp.tile([P, 7], f32)
            te.tensor_tensor(
                out=d7[:], in0=gat[:, 1:C], in1=tgx[:, 0:7], op=ALU.subtract
            )
            # |d| without tensor-tensor max (unsupported in the Pool Q7
            # library): ad = d * (1 - 2*[d<0])
            sg = sp.tile([P, 7], f32)
            te.tensor_scalar(
                out=sg[:], in0=d7[:], scalar1=0.0, scalar2=None, op0=ALU.is_lt
            )
            te.tensor_scalar(
                out=sg[:], in0=sg[:], scalar1=-2.0, scalar2=1.0,
                op0=ALU.mult, op1=ALU.add,
            )
            ad = sp.tile([P, 7], f32)
            te.tensor_tensor(out=ad[:], in0=d7[:], in1=sg[:], op=ALU.mult)
            q = sp.tile([P, 7], f32)
            te.tensor_tensor(out=q[:], in0=ad[:], in1=ad[:], op=ALU.mult)
            te.tensor_scalar(
                out=q[:], in0=q[:], scalar1=0.5, scalar2=None, op0=ALU.mult
            )
            lin = sp.tile([P, 7], f32)
            te.tensor_scalar(
                out=lin[:], in0=ad[:], scalar1=0.5, scalar2=None, op0=ALU.subtract
            )
            lt = sp.tile([P, 7], f32)
            te.tensor_scalar(
                out=lt[:], in0=ad[:], scalar1=1.0, scalar2=None, op0=ALU.is_lt
            )
            # branchless select: sl1 = lin + lt*(q - lin)
            df = sp.tile([P, 7], f32)
            te.tensor_tensor(out=df[:], in0=q[:], in1=lin[:], op=ALU.subtract)
            te.tensor_tensor(out=df[:], in0=df[:], in1=lt[:], op=ALU.mult)
            sl1 = sp.tile([P, 7], f32)
            te.tensor_tensor(out=sl1[:], in0=lin[:], in1=df[:], op=ALU.add)
            # row sum of 7 cols via an add tree (gpsimd has no free-axis reduce)
            t3 = sp.tile([P, 3], f32)
            te.tensor_tensor(
                out=t3[:], in0=sl1[:, 0:3], in1=sl1[:, 3:6], op=ALU.add
            )
            rs = sp.tile([P, 1], f32)
            te.tensor_tensor(
                out=rs[:], in0=t3[:, 0:1], in1=t3[:, 1:2], op=ALU.add
            )
            te.tensor_tensor(
                out=rs[:], in0=rs[:], in1=t3[:, 2:3], op=ALU.add
            )
            te.tensor_tensor(
                out=rs[:], in0=rs[:], in1=sl1[:, 6:7], op=ALU.add
            )
            te.tensor_tensor(
                out=out_t[:, COL_REG : COL_REG + 1], in0=rs[:], in1=m[:],
                op=ALU.mult,
            )

            nc.sync.dma_start(out=out_d[:], in_=out_t[:])

    nc.compile()
    return nc


def _host_indices(targets_core):
    """Replicate the device's f32 index math exactly: scale, clip, then
    round-to-nearest via the +-2^23 trick, all in float32."""
    t = targets_core.astype(np.float32)

    def grid(col, extent):
        g = np.minimum(t[:, col] * np.float32(extent / 80.0),
                       np.float32(extent - 1))
        g = np.maximum(g, np.float32(0.0))
        return (g + TWO23) - TWO23

    gx = grid(0, W)
    gy = grid(1, H)
    boff = np.repeat(np.arange(BPC, dtype=np.float32) * CELLS, N_TGT)
    return gy * np.float32(W) + gx + boff


def kernel(preds: np.ndarray, targets: np.ndarray) -> tuple:
    global _compiled
    preds = np.ascontiguousarray(np.asarray(preds, dtype=np.float32))
    targets = np.ascontiguousarray(np.asarray(targets, dtype=np.float32))

    # host-side layout prep: contiguous cls channel for the dense pass,
    # channel-last copy so one indirect-DMA row fetches all 8 channels of
    # a cell, and scatter addresses (index arithmetic on the tiny targets
    # tensor; all reductions happen on device).
    cls = np.ascontiguousarray(preds[:, 0])                       # (B,H,W)
    cl8 = np.ascontiguousarray(
        np.transpose(preds.reshape(B, C, CELLS), (0, 2, 1))       # (B,CELLS,C)
    )

    if _compiled is None:
        _compiled = _build()
    nc = _compiled

    slot_idx = np.arange(1, SLOTS + 1, dtype=np.float32)
    in_maps = []
    for c in range(NCORES):
        b0 = c * BPC
        tcore = targets[b0 : b0 + BPC].reshape(SLOTS, 7)
        fc = _host_indices(tcore)                                 # (128,) f32
        fci = fc.astype(np.int32)
        tgx = np.zeros((P, TGX_COLS), dtype=np.float32)
        tgx[:, 0:7] = tcore
        tgx[:, 7] = fc
        tgx[:, 8] = fci.view(np.float32)
        tgx[:, 9] = slot_idx
        frow = np.concatenate([fc, slot_idx]).reshape(1, 2 * P)
        in_maps.append({
            "cls": np.ascontiguousarray(cls[b0 : b0 + BPC].reshape(P, FREE)),
            "tgx": tgx,
            "frow": np.ascontiguousarray(frow),
            "fci": fci.reshape(P, 1),
            "cl8": cl8[b0 : b0 + BPC].reshape(CORE_ELEMS, C),
        })

    try:
        res = run_bass_kernel_spmd(nc, in_maps, list(range(NCORES))).results
    except Exception:
        # the axon worker occasionally dies with NRT_EXEC_UNIT_UNRECOVERABLE
        # on arbitrary ops and recovers on the next attempt; retry once.
        res = run_bass_kernel_spmd(nc, in_maps, list(range(NCORES))).results

    outs = np.stack([np.asarray(r["out"], dtype=np.float64) for r in res])
    s_softplus = (outs[:, :, COL_LN1].sum() + outs[:, :, COL_LN2].sum()
                  + outs[:, :, COL_LN3].sum())
    s_x = outs[:, :, COL_X].sum()
    num_objects = outs[:, :, COL_M].sum()
    s_reg = outs[:, :, COL_REG].sum()

    m_total = float(B * H * W)
    cls_loss = s_softplus / m_total - s_x / m_total
    reg_loss = s_reg / (num_objects + 1e-6) if num_objects > 0 else 0.0
    total = np.float32(cls_loss + 2.0 * reg_loss)
    return total, np.float32(num_objects)
